# revision 5
# baseline (speedup 1.0000x reference)
"""DGL capsule routing layer on 8 trn2 NeuronCores (Bass/Tile) — v5.

Math per iteration (b0 = 0):
    c = softmax(b, axis=out); s = einsum('io,iof->of', c, uh)
    v = squash(s); b += einsum('iof,of->io', uh, v)
Output: final v [OUT, F].

v5 = v4 (f-major bf16 host-side shard, 436us) + running-b reformulation:
    v_t = g_t[o] * s_t  with  g = |s|/(1+|s|^2)  (squash scale per o)
    b_t = b_{t-1} + g_{t-1}[o] * (sum_f uh[i,o,f] * s_{t-1}[o,f])
  so passes >= 2 consume the RAW AllReduce output s directly:
  * s_q broadcast tiles [128, QT] fill per-quarter right after each
    AR_q completes — overlapped with the remaining AR chain, instead
    of a post-squash 4MB w broadcast on the critical path.
  * the post-AR serial work shrinks to the [8,128] g-chain (one Sqrt
    table) + a 256KB g_rep broadcast.
  * b lives as 4 persistent [128,1024] bf16 tiles, updated in place:
    b += g_rep .* tree(uh .* s_bcast);  e = Exp(b) as before.
  * no w accumulation, no w_dram bounce, no srps/v_sb except in the
    final pass (output tail unchanged).
  * pass-1 AR split in 2 contiguous halves {q0,q1}, {q2,q3}: half-A
    s-broadcast overlaps the AR of half B.
  * the previous boundary's g-chain is emitted AFTER block 0's tree in
    the next pass so DVE program order lets block-0 tree muls pace the
    in-flight AR chain (q-mul k waits only AR_k + bcast).

Layout (unchanged from v4): cache col g = f*1024 + o; p-major [128,128]
view of f-major flat: p = f*8 + (o>>7), c = o&127; o-group m = p % 8.
"""

import numpy as np
from contextlib import ExitStack

import ml_dtypes

import concourse.bass as bass
import concourse.mybir as mybir
import concourse.tile as tile
from concourse import bacc
from concourse import bass_utils

F32 = mybir.dt.float32
BF16 = mybir.dt.bfloat16
AF = mybir.ActivationFunctionType
AO = mybir.AluOpType

IN_NODES, OUT_NODES, F_SIZE = 4096, 1024, 16
CORES = 8
I_LOC = IN_NODES // CORES          # 512 in-nodes per core
ROW = OUT_NODES * F_SIZE           # 16384 cache cols per in-node
P = 128
NBLK = I_LOC // P                  # 4 i-blocks per core
O = OUT_NODES
QT = ROW // 4                      # 4096 cols per f-quarter (4 f-planes)
NQ = 4
H = ROW // 2


def _body(nc, tc, uh, v_out, R, rg):
    uh_t = uh.rearrange("(n p) r -> n p r", p=P)   # [NBLK, 128, 16384] bf16

    with ExitStack() as ctx:
        persist = ctx.enter_context(tc.tile_pool(name="persist", bufs=1))
        scp = ctx.enter_context(tc.tile_pool(name="scp", bufs=1))
        smp = ctx.enter_context(tc.tile_pool(name="smp", bufs=1))
        psp = ctx.enter_context(tc.tile_pool(name="psp", bufs=3, space="PSUM"))
        psq = ctx.enter_context(tc.tile_pool(name="psq", bufs=2, space="PSUM"))
        dram = ctx.enter_context(tc.tile_pool(name="dram", bufs=2, space="DRAM"))

        # --- persistent tiles -------------------------------------------
        uhb = [persist.tile([P, ROW], BF16, name=f"uhb{k}", tag=f"uhb{k}")
               for k in range(NBLK)]
        s_qs = b_prev = g_rep = None
        if R > 1:
            s_qs = [persist.tile([P, QT], BF16, name=f"s_q{q}", tag=f"s_q{q}")
                    for q in range(NQ)]
            b_prev = [persist.tile([P, O], BF16, name=f"bp{k}", tag=f"bp{k}")
                      for k in range(NBLK)]
            g_rep = persist.tile([P, O], BF16, name="g_rep", tag="g_rep")
        c0 = persist.tile([P, 1], BF16, name="c0")
        nc.vector.memset(c0, 1.0 / OUT_NODES)
        # squash one-hots in the f-major p-mapping (o-group m = p % 8)
        pidx = np.arange(P)
        m_of_p = pidx % 8
        oh1_d = nc.inline_tensor(
            (m_of_p[:, None] == np.arange(8)[None, :]).astype('bfloat16'),
            name="oh1d")
        oh2_d = nc.inline_tensor(
            (np.arange(8)[:, None] == m_of_p[None, :]).astype('bfloat16'),
            name="oh2d")
        oh1 = persist.tile([P, 8], BF16, name="oh1")
        nc.sync.dma_start(oh1, oh1_d.ap())
        oh2 = persist.tile([8, P], BF16, name="oh2")
        nc.sync.dma_start(oh2, oh2_d.ap())

        def half_acc(blk, qa, qb):
            """acc[0:1024] = sum_f over quarters {qa,qb} of uh .* s."""
            acc = scp.tile([P, QT], BF16, tag="acc", name="acc")
            nc.vector.tensor_mul(acc, uhb[blk][:, qa * QT:(qa + 1) * QT],
                                 s_qs[qa])
            tmp = scp.tile([P, QT], BF16, tag="tmp", name="tmp")
            nc.vector.tensor_mul(tmp, uhb[blk][:, qb * QT:(qb + 1) * QT],
                                 s_qs[qb])
            nc.vector.tensor_add(acc, acc, tmp)
            nc.vector.tensor_add(acc[:, 0:2048], acc[:, 0:2048],
                                 acc[:, 2048:4096])
            return acc

        def tree_halfA(blk):
            """Park the {q0,q1} partial in the (currently idle) e2
            buffer of this block — runs while AR_2/3 are in flight."""
            acc = half_acc(blk, 0, 1)
            park = smp.tile([P, O], BF16, tag=f"e2_{blk}", name="park")
            nc.vector.tensor_add(park, acc[:, 0:O], acc[:, O:2 * O])
            return park

        def tree_halfB(blk, park):
            acc = half_acc(blk, 2, 3)
            nc.vector.tensor_add(acc[:, 0:O], acc[:, 0:O], acc[:, O:2 * O])
            d = scp.tile([P, O], BF16, tag="pt", bufs=2, name="d")
            nc.vector.tensor_add(d, park, acc[:, 0:O])
            return d

        def g_chain(sld_srcs):
            """sc[8,128] = |s|/(1+|s|^2) from the AR result quarters."""
            sld = smp.tile([P, P], BF16, tag="sld", bufs=2, name="sld")
            for src, sl in sld_srcs:
                nc.sync.dma_start(sld[sl, :], src)
            ssq = smp.tile([P, P], BF16, tag="ssq", name="ssq")
            nc.vector.tensor_mul(ssq, sld, sld)
            sqps = psq.tile([8, P], F32, tag="sqps", bufs=1, name="sqps")
            nc.tensor.matmul(sqps, oh1, ssq, start=True, stop=True,
                             skip_group_check=True)
            sq = smp.tile([8, P], BF16, tag="sq", name="sq")
            nc.scalar.copy(sq, sqps)
            y = smp.tile([8, P], BF16, tag="y", name="y")
            nc.scalar.activation(y, sq, AF.Sqrt)
            d1 = smp.tile([8, P], BF16, tag="t1", name="d1")
            nc.vector.tensor_scalar(d1, sq, 1.0, None, AO.add)
            rd = smp.tile([8, P], BF16, tag="ry", name="rd")
            with nc.allow_low_precision(reason="bf16 squash chain"):
                nc.vector.reciprocal(rd, d1)
            sc = smp.tile([8, P], BF16, tag="sq2", name="sc")
            nc.vector.tensor_mul(sc, y, rd)
            return sld, sc

        def gd_b_e2(blk, d, t):
            gd = scp.tile([P, O], BF16, tag="pt", bufs=2, name="gd")
            nc.vector.tensor_mul(gd, d, g_rep)
            if t == 2:
                nc.vector.tensor_copy(b_prev[blk], gd)
            else:
                nc.vector.tensor_add(b_prev[blk], b_prev[blk], gd)
            e2 = smp.tile([P, O], BF16, tag=f"e2_{blk}", name="e2")
            den = smp.tile([P, 1], F32, tag="den", name="den")
            nc.scalar.activation(e2, b_prev[blk], AF.Exp, accum_out=den)
            rinv = smp.tile([P, 1], F32, tag="rinv", name="rinv")
            nc.vector.reciprocal(rinv, den)
            rb = smp.tile([P, 1], BF16, tag=f"rb_{blk}", name="rb")
            nc.vector.tensor_copy(rb, rinv)
            return e2, rb

        pending = None   # sld_srcs of the boundary awaiting its g-chain

        for t in range(1, R + 1):
            final = (t == R)
            if t == 1:
                # two contiguous halves {q0,q1} and {q2,q3}
                ar_hin = [dram.tile([H], BF16, tag=f"arh_in{h}",
                                    name=f"ahi{h}") for h in range(2)]
                ar_hout = [dram.tile([H], BF16, tag=f"arh_out{h}",
                                     name=f"aho{h}") for h in range(2)]
                for fq in range(NQ):
                    sp = slice(fq * QT, (fq + 1) * QT)
                    for blk in range(NBLK):
                        nc.sync.dma_start(uhb[blk][:, sp], uh_t[blk, :, sp])
                for fq in range(NQ):
                    for sub in range(4):
                        ps = psp.tile([1, O], F32, tag="ps1", name="ps",
                                      padded_shape=[P, O])
                        base = fq * QT + sub * O
                        for blk in range(NBLK):
                            for w0 in (0, 512):
                                nc.tensor.matmul(
                                    ps[:, w0:w0 + 512], c0,
                                    uhb[blk][:, base + w0:base + w0 + 512],
                                    start=(blk == 0), stop=(blk == NBLK - 1),
                                    skip_group_check=True)
                        fl = smp.tile([1, O], BF16, tag="bfl", bufs=2,
                                      name="fl")
                        nc.scalar.copy(fl, ps)
                        nc.scalar.dma_start(
                            ar_hin[fq // 2][(fq % 2) * QT + sub * O:
                                            (fq % 2) * QT + (sub + 1) * O],
                            fl)
                    if fq % 2 == 1:
                        nc.gpsimd.collective_compute(
                            "AllReduce", AO.add, replica_groups=rg,
                            ins=[ar_hin[fq // 2].opt()],
                            outs=[ar_hout[fq // 2].opt()])
                ar_q = [ar_hout[q // 2][(q % 2) * QT:(q % 2 + 1) * QT]
                        for q in range(NQ)]
                sld_srcs = [(ar_hout[h].rearrange("(p c) -> p c", c=P),
                             slice(h * 64, (h + 1) * 64)) for h in range(2)]
            else:
                # ---- passes >= 2: ALL blocks' half-A trees (gated only
                # on s_q0/s_q1) pace the in-flight AR chain, then the
                # DEFERRED g-chain of the previous boundary, then half-B
                # + b-update per block.
                parks = [tree_halfA(blk) for blk in range(NBLK)]
                sld, sc = g_chain(pending)
                g_dram = dram.tile([O], BF16, tag="g_dram", name="g_dram")
                nc.sync.dma_start(
                    g_dram.rearrange("(p c) -> p c", p=8), sc)
                nc.sync.dma_start(
                    g_rep, g_dram[None, :].broadcast_to([P, O]))
                rbs = [None] * NBLK
                e2s = [None] * NBLK
                for blk in range(NBLK):
                    d = tree_halfB(blk, parks[blk])
                    e2s[blk], rbs[blk] = gd_b_e2(blk, d, t)

                # ---- s partials: fq-outer so AR_q fires early ----------
                ar_ins = [dram.tile([QT], BF16, tag=f"ar_in{q}",
                                    name=f"ari{q}") for q in range(NQ)]
                ar_outs = [dram.tile([QT], BF16, tag=f"ar_out{q}",
                                     name=f"aro{q}") for q in range(NQ)]
                for fq in range(NQ):
                    for sub in range(4):
                        base = fq * QT + sub * O
                        ps = psp.tile([1, O], F32, tag="ps1", name="ps",
                                      padded_shape=[P, O])
                        for blk in range(NBLK):
                            pt = scp.tile([P, O], BF16, tag="pt", bufs=2,
                                          name="pt")
                            nc.vector.tensor_mul(
                                pt, uhb[blk][:, base:base + O], e2s[blk])
                            for w0 in (0, 512):
                                nc.tensor.matmul(
                                    ps[:, w0:w0 + 512], rbs[blk],
                                    pt[:, w0:w0 + 512],
                                    start=(blk == 0), stop=(blk == NBLK - 1),
                                    skip_group_check=True)
                        fl = smp.tile([1, O], BF16, tag="bfl", bufs=2,
                                      name="fl")
                        nc.scalar.copy(fl, ps)
                        nc.scalar.dma_start(
                            ar_ins[fq][sub * O:(sub + 1) * O], fl)
                    nc.gpsimd.collective_compute(
                        "AllReduce", AO.add, replica_groups=rg,
                        ins=[ar_ins[fq].opt()], outs=[ar_outs[fq].opt()])
                ar_q = ar_outs
                sld_srcs = [(ar_outs[q].rearrange("(p c) -> p c", c=P),
                             slice(q * 32, (q + 1) * 32)) for q in range(NQ)]

            if not final:
                # broadcast raw s quarters as their ARs complete; the
                # g-chain for this boundary is deferred into pass t+1
                for q in range(NQ):
                    nc.sync.dma_start(
                        s_qs[q], ar_q[q][None, :].broadcast_to([P, QT]))
                pending = sld_srcs
            else:
                # output tail: full squash, v = sld .* bcast(sc)
                sld, sc = g_chain(sld_srcs)
                srps = psq.tile([P, P], F32, tag="srps", bufs=1, name="srps")
                nc.tensor.matmul(srps, oh2, sc, start=True, stop=True,
                                 skip_group_check=True)
                v_sb = smp.tile([P, P], BF16, tag="v_sb", name="v_sb")
                nc.vector.tensor_mul(v_sb, sld, srps)
                nc.sync.dma_start(v_out, v_sb)


def _build(routing_num: int):
    R = int(routing_num)
    assert R >= 1
    nc = bacc.Bacc(
        "TRN2", target_bir_lowering=False, debug=False, num_devices=CORES)
    uh = nc.dram_tensor("uh", [I_LOC, ROW], BF16, kind="ExternalInput")
    v_out = nc.dram_tensor("v_out", [P, P], BF16, kind="ExternalOutput")
    rg = [list(range(CORES))]
    with tile.TileContext(nc) as tc:
        _body(nc, tc, uh.ap(), v_out.ap(), R, rg)
    nc.compile()
    return nc


_CACHE: dict = {}


def _get_nc(routing_num: int):
    R = int(routing_num)
    if R not in _CACHE:
        _CACHE[R] = _build(R)
    return _CACHE[R]


def _shard(u_hat: np.ndarray):
    uh = np.asarray(u_hat, dtype=np.float32)
    assert uh.shape == (IN_NODES * OUT_NODES, F_SIZE), uh.shape
    # per core: [512, 1024, 16] -> f-major [512, 16, 1024] bf16
    uh = uh.reshape(IN_NODES, OUT_NODES, F_SIZE)
    out = []
    for k in range(CORES):
        blkk = uh[k * I_LOC:(k + 1) * I_LOC]          # [512, 1024, 16]
        fm = np.ascontiguousarray(blkk.transpose(0, 2, 1)).reshape(I_LOC, ROW)
        out.append({"uh": fm.astype(ml_dtypes.bfloat16)})
    return out


def run(u_hat, routing_num, trace=False):
    nc = _get_nc(routing_num)
    in_maps = _shard(u_hat)
    res = bass_utils.run_bass_kernel_spmd(
        nc, in_maps, core_ids=list(range(CORES)), trace=trace)
    return res


def _unpack(v_pm) -> np.ndarray:
    # [128,128] p-major bf16, p = f*8 + (o>>7), c = o & 127
    v = np.asarray(v_pm).astype(np.float32).reshape(F_SIZE, 8, P)
    return np.ascontiguousarray(
        v.transpose(1, 2, 0).reshape(OUT_NODES, F_SIZE))


def kernel(u_hat, routing_num):
    res = run(u_hat, routing_num, trace=False)
    return _unpack(res.results[0]["v_out"])
